# revision 18
# baseline (speedup 1.0000x reference)
"""nn_GPT_64347200029289 — RWKV6-style transformer on 8 trn2 cores.

Sharding: DP=4 over batch x TP=2 over heads / FFN. core = 2*batch + rank.
Activations feature-major [C, T] on-chip; per-pair AllReduce after Wo and
after cmix; lm_head sharded over V across all 8 cores via AllGather of the
final token.
"""
import os
import numpy as np
import ml_dtypes

C, H, L, V = 768, 12, 12, 50304
N = C // H          # 64
B, T, Q = 4, 1024, 256
NCH = T // Q        # 4 chunks
GN_EPS = 1e-5 * 64
LN_EPS = 1e-5
NCORES = 8
HPC = H // 2        # heads per core (6)
FS = HPC * N        # 384 tmix shard
F3 = 3 * C // 2     # 1152 cmix shard
VS = V // NCORES    # 6288
PAIRS = [[0, 1], [2, 3], [4, 5], [6, 7]]
GATHER_GROUPS = [[0, 2, 4, 6], [1, 3, 5, 7]]
BF16 = np.float16
AB16 = ml_dtypes.bfloat16  # attention-block dtype

# ---- params column layout (per-partition f32 [128, PCOLS]) ----
_c = 0
def _cols(n):
    global _c
    s = _c
    _c += n
    return s
IMAA = _cols(24)    # (mix 4)x(ct 6): 1-maa
AMAA = _cols(24)    # maa
ICMA = _cols(12)    # (mix 2)x(ct 6): 1-cmaa
ACMA = _cols(12)
BR = _cols(3)
BK = _cols(3)
BCK = _cols(9)
BCR = _cols(6)
BO = _cols(6)
BCV = _cols(6)
GNW = _cols(3)
WB = _cols(12)      # (ih 2)x(h 6)
WK = _cols(12)      # (jh 2)x(h 6)
WS = _cols(6)       # rows 0..63
PCOLS = _c

_g_cache = {}


def _host_pack(inputs):
    f = lambda a: np.ascontiguousarray(np.asarray(a, np.float32))
    idx = np.asarray(inputs["idx"]).astype(np.int64)
    wte, wpe = f(inputs["wte"]), f(inputs["wpe"])
    for k in ("bv", "bg", "gn_b", "bcv", "bck", "ln1_b", "ln2_b"):
        assert np.allclose(np.asarray(inputs[k]), 0.0), f"{k} must be 0"

    Wr, Wk_, Wv, Wg = f(inputs["Wr"]), f(inputs["Wk"]), f(inputs["Wv"]), f(inputs["Wg"])
    Wo, Wck, Wcv, Wcr = f(inputs["Wo"]), f(inputs["Wck"]), f(inputs["Wcv"]), f(inputs["Wcr"])

    td, tf = f(inputs["tdecay"]), f(inputs["tfaaaa"])  # [L,H]
    w = np.exp(-np.exp(td.astype(np.float64)))
    ii = np.arange(Q)
    diff = ii[:, None] - ii[None, :]
    e = np.where(diff > 0, diff - 1, 0).astype(np.float64)
    Wmat = np.where(diff[None, None] > 0, w[:, :, None, None] ** e[None, None],
                    np.where(diff[None, None] == 0, tf[:, :, None, None].astype(np.float64), 0.0))
    WmatT = Wmat.transpose(0, 1, 3, 2)                  # [L,H,j,i]
    wk_d = w[:, :, None] ** (Q - 1 - ii)[None, None, :]  # [L,H,Q]
    wb_d = w[:, :, None] ** ii[None, None, :]            # [L,H,Q]
    ws_d = (w ** Q).astype(np.float32)                   # [L,H]

    per_rank = {}
    for rank in range(2):
        hs = slice(rank * FS, (rank + 1) * FS)
        fs = slice(rank * F3, (rank + 1) * F3)
        heads = list(range(rank * HPC, (rank + 1) * HPC))

        wrk = np.empty((L, 128, 6, 2 * FS), BF16)
        wvg = np.empty((L, 128, 6, 2 * FS), BF16)
        wo = np.empty((L, 128, 3, C), AB16)
        wck = np.empty((L, 128, 6, F3), BF16)
        wcv = np.empty((L, 128, 9, C), BF16)
        wcr = np.empty((L, 128, 6, C), BF16)
        wmt = np.empty((L, HPC, 128, 2, Q), AB16)
        params = np.zeros((L, 128, PCOLS), np.float32)
        prow = np.zeros((L, 1, 4 * C), np.float32)
        for l in range(L):
            for kt in range(6):
                rows = slice(kt * 128, (kt + 1) * 128)
                wrk[l, :, kt, :FS] = Wr[l][rows, hs]
                wrk[l, :, kt, FS:] = Wk_[l][rows, hs]
                wvg[l, :, kt, :FS] = Wv[l][rows, hs]
                wvg[l, :, kt, FS:] = Wg[l][rows, hs]
                wck[l, :, kt, :] = Wck[l][rows, fs]
                wcr[l, :, kt, :] = Wcr[l][rows, :]
            for kt in range(3):
                wo[l, :, kt, :] = Wo[l][rank * FS + kt * 128:rank * FS + (kt + 1) * 128, :]
            for kt in range(9):
                wcv[l, :, kt, :] = Wcv[l][rank * F3 + kt * 128:rank * F3 + (kt + 1) * 128, :]
            for hi, h in enumerate(heads):
                for jh in range(2):
                    wmt[l, hi, :, jh, :] = WmatT[l, h, jh * 128:(jh + 1) * 128, :]
            maas = [f(inputs["maa_tk"])[l], f(inputs["maa_tv"])[l],
                    f(inputs["maa_tr"])[l], f(inputs["maa_tg"])[l]]
            cmaas = [f(inputs["cmaa_k"])[l], f(inputs["cmaa_r"])[l]]
            for mi in range(4):
                for ct in range(6):
                    col = maas[mi][ct * 128:(ct + 1) * 128]
                    params[l, :, IMAA + mi * 6 + ct] = 1.0 - col
                    params[l, :, AMAA + mi * 6 + ct] = col
            for mi in range(2):
                for ct in range(6):
                    col = cmaas[mi][ct * 128:(ct + 1) * 128]
                    params[l, :, ICMA + mi * 6 + ct] = 1.0 - col
                    params[l, :, ACMA + mi * 6 + ct] = col
            br, bk = f(inputs["br"])[l][hs], f(inputs["bk"])[l][hs]
            for ft in range(3):
                params[l, :, BR + ft] = br[ft * 128:(ft + 1) * 128]
                params[l, :, BK + ft] = bk[ft * 128:(ft + 1) * 128]
                params[l, :, GNW + ft] = f(inputs["gn_w"])[l][hs][ft * 128:(ft + 1) * 128]
            bck = f(inputs["bck"])[l][fs]
            for ft in range(9):
                params[l, :, BCK + ft] = bck[ft * 128:(ft + 1) * 128]
            bcr = f(inputs["bcr"])[l]
            bo = f(inputs["bo"])[l] if rank == 0 else np.zeros(C, np.float32)
            bcv = f(inputs["bcv"])[l] if rank == 0 else np.zeros(C, np.float32)
            for ct in range(6):
                params[l, :, BCR + ct] = bcr[ct * 128:(ct + 1) * 128]
                params[l, :, BO + ct] = bo[ct * 128:(ct + 1) * 128]
                params[l, :, BCV + ct] = bcv[ct * 128:(ct + 1) * 128]
            for hi, h in enumerate(heads):
                for ih in range(2):
                    params[l, :, WB + ih * 6 + hi] = wb_d[l, h, ih * 128:(ih + 1) * 128]
                    params[l, :, WK + ih * 6 + hi] = wk_d[l, h, ih * 128:(ih + 1) * 128]
                params[l, :, WS + hi] = ws_d[l, h]
            prow[l, 0, 0 * C:1 * C] = f(inputs["ln1_w"])[l]
            prow[l, 0, 1 * C:2 * C] = f(inputs["ln1_b"])[l]
            prow[l, 0, 2 * C:3 * C] = f(inputs["ln2_w"])[l]
            prow[l, 0, 3 * C:4 * C] = f(inputs["ln2_b"])[l]
        per_rank[rank] = dict(wrk=wrk, wvg=wvg, wo=wo, wck=wck, wcv=wcv,
                              wcr=wcr, wmt=wmt, params=params, prow=prow)

    lnfp = np.zeros((128, 6, 2), np.float32)
    for ct in range(6):
        lnfp[:, ct, 0] = f(inputs["lnf_w"])[ct * 128:(ct + 1) * 128]
        lnfp[:, ct, 1] = f(inputs["lnf_b"])[ct * 128:(ct + 1) * 128]

    in_maps = []
    for core in range(NCORES):
        b, rank = core // 2, core % 2
        x0 = (wte[idx[b]] + wpe[:T]).T                  # [C, T] f32
        x0p = np.zeros((128, 6, T + 1), np.float32)
        for ct in range(6):
            x0p[:, ct, 1:] = x0[ct * 128:(ct + 1) * 128]
        wteT = np.empty((128, 6, VS), BF16)
        vsl = slice(core * VS, (core + 1) * VS)
        for kt in range(6):
            wteT[:, kt, :] = wte[vsl, kt * 128:(kt + 1) * 128].T
        m = dict(per_rank[rank])
        m["x0"] = x0p
        m["wteT"] = wteT
        m["lnfp"] = lnfp
        in_maps.append(m)
    return in_maps


def _build():
    import concourse.bass as bass
    import concourse.mybir as mybir
    import concourse.tile as tile
    from concourse import bacc
    from concourse.masks import make_identity
    from contextlib import ExitStack

    dt = mybir.dt
    AF = mybir.ActivationFunctionType
    ALU = mybir.AluOpType
    F32, B16 = dt.float32, dt.float16
    A16 = dt.bfloat16

    nc = bacc.Bacc("TRN2", target_bir_lowering=False, debug=False,
                   num_devices=NCORES)
    di = {}
    def dram_in(name, shape, dtype):
        di[name] = nc.dram_tensor(name, list(shape), dtype, kind="ExternalInput")
        return di[name]

    dram_in("x0", [128, 6, T + 1], F32)
    dram_in("wrk", [L, 128, 6, 2 * FS], B16)
    dram_in("wvg", [L, 128, 6, 2 * FS], B16)
    dram_in("wo", [L, 128, 3, C], A16)
    dram_in("wck", [L, 128, 6, F3], B16)
    dram_in("wcv", [L, 128, 9, C], B16)
    dram_in("wcr", [L, 128, 6, C], B16)
    dram_in("wmt", [L, HPC, 128, 2, Q], A16)
    dram_in("params", [L, 128, PCOLS], F32)
    dram_in("prow", [L, 1, 4 * C], F32)
    dram_in("wteT", [128, 6, VS], B16)
    dram_in("lnfp", [128, 6, 2], F32)
    logits = nc.dram_tensor("logits", [4, VS], F32, kind="ExternalOutput")

    with tile.TileContext(nc) as tc:
        with ExitStack() as ctx:
            consts = ctx.enter_context(tc.tile_pool(name="consts", bufs=1))
            resid = ctx.enter_context(tc.tile_pool(name="resid", bufs=1))
            wbig = ctx.enter_context(tc.tile_pool(name="wbig", bufs=1))
            wpc = ctx.enter_context(tc.tile_pool(name="wpc", bufs=2))
            lnp = ctx.enter_context(tc.tile_pool(name="lnp", bufs=1))
            stream = ctx.enter_context(tc.tile_pool(name="stream", bufs=2))
            mixp = ctx.enter_context(tc.tile_pool(name="mixp", bufs=2))
            projp = ctx.enter_context(tc.tile_pool(name="projp", bufs=5))
            att = ctx.enter_context(tc.tile_pool(name="att", bufs=2))
            attk = ctx.enter_context(tc.tile_pool(name="attk", bufs=2))
            statep = ctx.enter_context(tc.tile_pool(name="statep", bufs=1))
            smalls = ctx.enter_context(tc.tile_pool(name="smalls", bufs=2))
            lnst = ctx.enter_context(tc.tile_pool(name="lnst", bufs=1))
            xxp = ctx.enter_context(tc.tile_pool(name="xxp", bufs=6))
            h2p = ctx.enter_context(tc.tile_pool(name="h2p", bufs=1))
            headp = ctx.enter_context(tc.tile_pool(name="headp", bufs=1))
            psum = ctx.enter_context(tc.tile_pool(name="psum", bufs=2, space="PSUM"))
            psum1 = ctx.enter_context(tc.tile_pool(name="psum1", bufs=1, space="PSUM"))
            dram = ctx.enter_context(tc.tile_pool(name="dram", bufs=2, space="DRAM"))

            ones_bf = consts.tile([128, 1], B16)
            nc.vector.memset(ones_bf, 1.0)
            ones_row = consts.tile([1, 512], F32)
            nc.vector.memset(ones_row, 1.0)
            eps_ln = consts.tile([1, 1], F32)
            nc.vector.memset(eps_ln, LN_EPS)
            eps_gn = consts.tile([128, 1], F32)
            nc.vector.memset(eps_gn, GN_EPS)
            eps_f = consts.tile([4, 1], F32)
            nc.vector.memset(eps_f, LN_EPS)
            ident_bf = consts.tile([128, 128], A16)
            make_identity(nc, ident_bf[:])
            ident_f4 = consts.tile([4, 4], F32)
            make_identity(nc, ident_f4[:])

            x = resid.tile([128, 6, T + 1], F32)
            nc.sync.dma_start(out=x, in_=di["x0"].ap())

            def layer_norm(pm, l, seg, nmix, imaa0, amaa0):
                """LN of x -> xln [128,6,T+1] bf16 (col0=0) + nmix mix tiles."""
                pr = wpc.tile([1, 2 * C], F32, tag="prow")
                nc.sync.dma_start(out=pr, in_=di["prow"].ap()[l, :, seg * 2 * C:(seg + 1) * 2 * C])
                goff, boff = 0, C
                xln = lnp.tile([128, 6, T + 1], B16, tag="xln")
                nc.vector.memset(xln[:, :, 0:1], 0.0)
                stats = psum1.tile([65, 2, 512], F32, tag="stats")  # sum@p0, sumsq@p64
                for kt in range(6):
                    xbf = stream.tile([128, T], B16, tag="xbf")
                    xsq = stream.tile([128, T], B16, tag="xsq")
                    nc.scalar.activation(xbf, x[:, kt, 1:], AF.Copy)
                    nc.scalar.activation(xsq, x[:, kt, 1:], AF.Square)
                    for tt in range(2):
                        ts = slice(tt * 512, (tt + 1) * 512)
                        nc.tensor.matmul(stats[0:1, tt, :], ones_bf, xbf[:, ts],
                                         start=(kt == 0), stop=(kt == 5))
                        nc.tensor.matmul(stats[64:65, tt, :], ones_bf, xsq[:, ts],
                                         start=(kt == 0), stop=(kt == 5))
                rstd = lnst.tile([1, T], F32, tag="rstd")
                nmurs = lnst.tile([1, T], F32, tag="nmurs")
                for tt in range(2):
                    ts = slice(tt * 512, (tt + 1) * 512)
                    mu_r = smalls.tile([1, 512], F32, tag="st_a")
                    nc.scalar.activation(mu_r, stats[0:1, tt, :], AF.Copy, scale=1.0 / C)
                    msq_r = smalls.tile([1, 512], F32, tag="st_b")
                    nc.scalar.activation(msq_r, stats[64:65, tt, :], AF.Copy, scale=1.0 / C)
                    mu2 = smalls.tile([1, 512], F32, tag="st_a")
                    nc.vector.tensor_tensor(mu2, mu_r, mu_r, ALU.mult)
                    var = smalls.tile([1, 512], F32, tag="st_b")
                    nc.vector.tensor_tensor(var, msq_r, mu2, ALU.subtract)
                    sd = smalls.tile([1, 512], F32, tag="st_b")
                    nc.scalar.activation(sd, var, AF.Sqrt, bias=eps_ln[:])
                    nc.vector.reciprocal(rstd[:, ts], sd)
                    mrs = smalls.tile([1, 512], F32, tag="st_b")
                    nc.vector.tensor_tensor(mrs, mu_r, rstd[:, ts], ALU.mult)
                    nc.vector.tensor_scalar(nmurs[:, ts], mrs, -1.0, None, ALU.mult)
                for ct in range(6):
                    for tt in range(2):
                        ts = slice(tt * 512, (tt + 1) * 512)
                        ts1 = slice(1 + tt * 512, 1 + (tt + 1) * 512)
                        bca = psum1.tile([128, 512], F32, tag="bca")
                        nc.tensor.matmul(bca, pr[0:1, goff + ct * 128:goff + (ct + 1) * 128],
                                         rstd[:, ts], start=True, stop=True)
                        bcb = psum1.tile([128, 512], F32, tag="bcb")
                        nc.tensor.matmul(bcb, pr[0:1, goff + ct * 128:goff + (ct + 1) * 128],
                                         nmurs[:, ts], start=True, stop=True)
                        nc.vector.tensor_tensor(xln[:, ct, ts1], x[:, ct, ts1], bca, ALU.mult)
                        nc.vector.tensor_tensor(xln[:, ct, ts1], xln[:, ct, ts1], bcb, ALU.add)
                xxs = []
                for ct in range(6):
                    xx = xxp.tile([128, T], B16, tag="xx")
                    nc.vector.tensor_tensor(xx, xln[:, ct, 0:T], xln[:, ct, 1:T + 1],
                                            ALU.subtract)
                    xxs.append(xx)
                mixes = []
                for mi in range(nmix):
                    mx = mixp.tile([128, 6, T], B16, tag="mix")
                    for ct in range(6):
                        nc.vector.scalar_tensor_tensor(
                            mx[:, ct, :], xxs[ct],
                            pm[:, amaa0 + mi * 6 + ct:amaa0 + mi * 6 + ct + 1],
                            xln[:, ct, 1:T + 1], ALU.mult, ALU.add)
                    mixes.append(mx)
                return mixes

            for l in range(L):
                pm = wpc.tile([128, PCOLS], F32, tag="params")
                nc.sync.dma_start(out=pm, in_=di["params"].ap()[l])

                # ---------------- tmix ----------------
                xk, xv, xr, xg = layer_norm(pm, l, 0, 4, IMAA, AMAA)

                rT = projp.tile([128, 3, T], A16, tag="proj")
                kT = projp.tile([128, 3, T], A16, tag="proj")
                for dst, mx, woff, boff in ((kT, xk, FS, BK), (rT, xr, 0, BR)):
                    for ft in range(3):
                        wp = wpc.tile([128, 6, 128], B16, tag="wtm")
                        nc.sync.dma_start(
                            out=wp, in_=di["wrk"].ap()[l, :, :, woff + ft * 128:woff + (ft + 1) * 128])
                        for tt in range(2):
                            ts = slice(tt * 512, (tt + 1) * 512)
                            ps = psum.tile([128, 512], F32, tag="proj")
                            for kt in range(6):
                                nc.tensor.matmul(ps, wp[:, kt, :], mx[:, kt, ts],
                                                 start=(kt == 0), stop=(kt == 5))
                            nc.scalar.activation(dst[:, ft, ts], ps, AF.Identity,
                                                 bias=pm[:, boff + ft:boff + ft + 1])
                v = projp.tile([128, 8, FS], A16, tag="proj")
                g = projp.tile([128, 8, FS], A16, tag="proj")
                w_vg = wbig.tile([128, 6, 2 * FS], B16, tag="wvg")
                nc.sync.dma_start(out=w_vg, in_=di["wvg"].ap()[l])
                for dst, mx, woff, fn in ((v, xv, 0, AF.Copy), (g, xg, FS, AF.Silu)):
                    for pt in range(8):
                        ps = psum.tile([128, FS], F32, tag="proj")
                        for kt in range(6):
                            nc.tensor.matmul(ps, mx[:, kt, pt * 128:(pt + 1) * 128],
                                             w_vg[:, kt, woff:woff + FS],
                                             start=(kt == 0), stop=(kt == 5))
                        nc.scalar.activation(dst[:, pt, :], ps, fn)

                # attention
                state_f = statep.tile([128, HPC // 2, N], F32, tag="state_f")
                state_b = statep.tile([128, HPC // 2, N], A16, tag="state_b")
                y = projp.tile([128, 8, FS], A16, tag="proj")
                for hi in range(HPC):
                    wm = att.tile([128, 2, Q], A16, tag="wmt")
                    nc.sync.dma_start(out=wm, in_=di["wmt"].ap()[l, hi])
                    po = (hi % 2) * 64
                    fq = hi // 2
                    for c in range(NCH):
                        cs = slice(c * Q, (c + 1) * Q)
                        att_ps = psum.tile([128, 2, Q], F32, tag="apsum")
                        for jh in range(2):
                            nc.tensor.matmul(
                                att_ps[:, jh],
                                kT[po:po + 64, fq, c * Q + jh * 128:c * Q + (jh + 1) * 128],
                                rT[po:po + 64, fq, cs], start=True, stop=True)
                        aw = attk.tile([128, 2, Q], A16, tag="aw")
                        nc.vector.tensor_tensor(aw, att_ps, wm, ALU.mult)
                        for ih in range(2):
                            pt = c * 2 + ih
                            y1 = psum.tile([128, N], F32, tag="apsum")
                            for jh in range(2):
                                nc.tensor.matmul(y1, aw[:, jh, ih * 128:(ih + 1) * 128],
                                                 v[:, c * 2 + jh, hi * N:(hi + 1) * N],
                                                 start=(jh == 0), stop=(jh == 1))
                            if c == 0:
                                nc.scalar.activation(y[:, pt, hi * N:(hi + 1) * N], y1, AF.Copy)
                            else:
                                y2 = psum.tile([128, N], F32, tag="apsum")
                                nc.tensor.matmul(
                                    y2, rT[po:po + 64, fq, c * Q + ih * 128:c * Q + (ih + 1) * 128],
                                    state_b[po:po + 64, fq, :], start=True, stop=True)
                                y2s = attk.tile([128, N], F32, tag="y2s")
                                nc.vector.tensor_scalar(y2s, y2, pm[:, WB + ih * 6 + hi:WB + ih * 6 + hi + 1],
                                                        None, ALU.mult)
                                nc.vector.tensor_tensor(y[:, pt, hi * N:(hi + 1) * N],
                                                        y1, y2s, ALU.add)
                        if c < NCH - 1:
                            kwks = []
                            for jh in range(2):
                                tr = psum.tile([128, N], A16, tag="apsum")
                                nc.tensor.transpose(
                                    tr, kT[po:po + 64, fq, c * Q + jh * 128:c * Q + (jh + 1) * 128],
                                    ident_bf[po:po + 64, po:po + 64])
                                kwk = attk.tile([128, N], A16, tag="kwk")
                                nc.vector.tensor_scalar(kwk, tr, pm[:, WK + jh * 6 + hi:WK + jh * 6 + hi + 1],
                                                        None, ALU.mult)
                                kwks.append(kwk)
                            st_ps = psum.tile([64, N], F32, tag="apsum")
                            for jh in range(2):
                                nc.tensor.matmul(st_ps, kwks[jh], v[:, c * 2 + jh, hi * N:(hi + 1) * N],
                                                 start=(jh == 0), stop=(jh == 1))
                            if c == 0:
                                nc.scalar.activation(state_f[po:po + 64, fq, :], st_ps, AF.Copy)
                            else:
                                sdec = attk.tile([128, N], F32, tag="sdec")
                                nc.vector.tensor_scalar(sdec[po:po + 64], state_f[po:po + 64, fq, :],
                                                        pm[po:po + 64, WS + hi:WS + hi + 1], None, ALU.mult)
                                nc.vector.tensor_tensor(state_f[po:po + 64, fq, :], sdec[po:po + 64], st_ps, ALU.add)
                            nc.scalar.activation(state_b[po:po + 64, fq, :], state_f[po:po + 64, fq, :], AF.Copy)

                # groupnorm (token-major) + gate
                for pt in range(8):
                    y3 = y[:, pt, :].rearrange("p (h n) -> p h n", h=HPC)
                    ysq = stream.tile([128, HPC, N], A16, tag="ysq")
                    nc.scalar.activation(ysq, y3, AF.Square)
                    ssum = smalls.tile([128, HPC], F32, tag="ssum")
                    nc.vector.tensor_reduce(ssum, y3, mybir.AxisListType.X, ALU.add)
                    ssq = smalls.tile([128, HPC], F32, tag="ssq")
                    nc.vector.tensor_reduce(ssq, ysq, mybir.AxisListType.X, ALU.add)
                    mu = smalls.tile([128, HPC], F32, tag="gmu")
                    nc.scalar.activation(mu, ssum, AF.Copy, scale=1.0 / N)
                    msq = smalls.tile([128, HPC], F32, tag="gmsq")
                    nc.scalar.activation(msq, ssq, AF.Copy, scale=1.0 / N)
                    mu2 = smalls.tile([128, HPC], F32, tag="gmu2")
                    nc.vector.tensor_tensor(mu2, mu, mu, ALU.mult)
                    var = smalls.tile([128, HPC], F32, tag="gvar")
                    nc.vector.tensor_tensor(var, msq, mu2, ALU.subtract)
                    sd = smalls.tile([128, HPC], F32, tag="gsd")
                    nc.scalar.activation(sd, var, AF.Sqrt, bias=eps_gn[:])
                    rstd = smalls.tile([128, HPC], F32, tag="grstd")
                    nc.vector.reciprocal(rstd, sd)
                    for hi in range(HPC):
                        nc.vector.tensor_scalar(y[:, pt, hi * N:(hi + 1) * N],
                                                y[:, pt, hi * N:(hi + 1) * N],
                                                mu[:, hi:hi + 1], rstd[:, hi:hi + 1],
                                                ALU.subtract, ALU.mult)
                    nc.vector.tensor_tensor(y[:, pt, :], y[:, pt, :], g[:, pt, :], ALU.mult)

                # transpose yg -> ygT with gn_w fold
                ygT = mixp.tile([128, 3, T], A16, tag="mix")
                for ft in range(3):
                    for pt in range(8):
                        tp = psum.tile([128, 128], A16, tag="apsum")
                        nc.tensor.transpose(tp, y[:, pt, ft * 128:(ft + 1) * 128], ident_bf[:])
                        nc.scalar.activation(ygT[:, ft, pt * 128:(pt + 1) * 128], tp,
                                             AF.Copy, scale=pm[:, GNW + ft:GNW + ft + 1])

                # Wo partial + AllReduce
                ar_ins = [dram.tile([128, 6, 512], B16, tag="ar_in", name=f"ari{_t}") for _t in range(2)]
                ar_outs = [dram.tile([128, 6, 512], B16, tag="ar_out", name=f"aro{_t}") for _t in range(2)]
                for tt in range(2):
                    ts = slice(tt * 512, (tt + 1) * 512)
                    for ct in range(6):
                        wp = wpc.tile([128, 3, 128], A16, tag="wo")
                        nc.sync.dma_start(out=wp, in_=di["wo"].ap()[l, :, :, ct * 128:(ct + 1) * 128])
                        ps = psum.tile([128, 512], F32, tag="proj")
                        for kt in range(3):
                            nc.tensor.matmul(ps, wp[:, kt, :], ygT[:, kt, ts],
                                             start=(kt == 0), stop=(kt == 2))
                        ot = stream.tile([128, 512], B16, tag="arin")
                        nc.scalar.activation(ot, ps, AF.Identity, bias=pm[:, BO + ct:BO + ct + 1])
                        nc.sync.dma_start(out=ar_ins[tt][:, ct, :], in_=ot)
                    nc.gpsimd.collective_compute(
                        "AllReduce", ALU.add, replica_groups=PAIRS,
                        ins=[ar_ins[tt].opt()], outs=[ar_outs[tt].opt()])
                for tt in range(2):
                    ts1 = slice(1 + tt * 512, 1 + (tt + 1) * 512)
                    for ct in range(6):
                        art = stream.tile([128, 512], B16, tag="arb")
                        nc.sync.dma_start(out=art, in_=ar_outs[tt][:, ct, :])
                        nc.vector.tensor_tensor(x[:, ct, ts1], x[:, ct, ts1], art, ALU.add)

                # ---------------- cmix ----------------
                xk2, xr2 = layer_norm(pm, l, 1, 2, ICMA, ACMA)
                h2 = h2p.tile([128, 9, T], B16, tag="h2")
                for ft in range(9):
                    wp = wpc.tile([128, 6, 128], B16, tag="wck")
                    nc.sync.dma_start(out=wp, in_=di["wck"].ap()[l, :, :, ft * 128:(ft + 1) * 128])
                    for tt in range(2):
                        ts = slice(tt * 512, (tt + 1) * 512)
                        ps = psum.tile([128, 512], F32, tag="proj")
                        for kt in range(6):
                            nc.tensor.matmul(ps, wp[:, kt, :], xk2[:, kt, ts],
                                             start=(kt == 0), stop=(kt == 5))
                        hr = stream.tile([128, 512], B16, tag="hr")
                        nc.scalar.activation(hr, ps, AF.Relu)
                        nc.vector.tensor_tensor(h2[:, ft, ts], hr, hr, ALU.mult)
                ar_ins2 = [dram.tile([128, 6, 512], B16, tag="ar_in", name=f"ari2{_t}") for _t in range(2)]
                ar_outs2 = [dram.tile([128, 6, 512], B16, tag="ar_out", name=f"aro2{_t}") for _t in range(2)]
                for tt in range(2):
                    ts = slice(tt * 512, (tt + 1) * 512)
                    for ct in range(6):
                        wpr = wpc.tile([128, 6, 128], B16, tag="wcr")
                        nc.sync.dma_start(out=wpr, in_=di["wcr"].ap()[l, :, :, ct * 128:(ct + 1) * 128])
                        wpv = wpc.tile([128, 9, 128], B16, tag="wcv")
                        nc.sync.dma_start(out=wpv, in_=di["wcv"].ap()[l, :, :, ct * 128:(ct + 1) * 128])
                        gp = psum.tile([128, 512], F32, tag="proj")
                        for kt in range(6):
                            nc.tensor.matmul(gp, wpr[:, kt, :], xr2[:, kt, ts],
                                             start=(kt == 0), stop=(kt == 5))
                        gt = stream.tile([128, 512], B16, tag="gate")
                        nc.scalar.activation(gt, gp, AF.Sigmoid, bias=pm[:, BCR + ct:BCR + ct + 1])
                        cp = psum.tile([128, 512], F32, tag="proj")
                        for kt in range(9):
                            nc.tensor.matmul(cp, wpv[:, kt, :], h2[:, kt, ts],
                                             start=(kt == 0), stop=(kt == 8))
                        ot = stream.tile([128, 512], B16, tag="arin")
                        nc.vector.tensor_tensor(ot, cp, gt, ALU.mult)
                        nc.sync.dma_start(out=ar_ins2[tt][:, ct, :], in_=ot)
                    nc.gpsimd.collective_compute(
                        "AllReduce", ALU.add, replica_groups=PAIRS,
                        ins=[ar_ins2[tt].opt()], outs=[ar_outs2[tt].opt()])
                for tt in range(2):
                    ts1 = slice(1 + tt * 512, 1 + (tt + 1) * 512)
                    for ct in range(6):
                        art = stream.tile([128, 512], B16, tag="arb")
                        nc.sync.dma_start(out=art, in_=ar_outs2[tt][:, ct, :])
                        nc.vector.tensor_tensor(x[:, ct, ts1], x[:, ct, ts1], art, ALU.add)

            # ---------------- head ----------------
            xl_d = dram.tile([6, 128], F32, tag="xl")
            for ct in range(6):
                nc.sync.dma_start(out=xl_d[ct], in_=x[:, ct, T])
            xg_d = dram.tile([4, C], F32, tag="xg")
            nc.gpsimd.collective_compute(
                "AllGather", mybir.AluOpType.bypass, replica_groups=GATHER_GROUPS,
                ins=[xl_d.opt()], outs=[xg_d.opt()])
            x4 = headp.tile([4, C], F32, tag="x4")
            nc.sync.dma_start(out=x4, in_=xg_d[:])
            s4 = headp.tile([4, 1], F32, tag="s4")
            nc.vector.tensor_reduce(s4, x4, mybir.AxisListType.X, ALU.add)
            trash = headp.tile([4, C], B16, tag="out_t")
            sq4 = headp.tile([4, 1], F32, tag="sq4")
            nc.scalar.activation(trash, x4, AF.Square, accum_out=sq4)
            mu4 = headp.tile([4, 1], F32, tag="mu4")
            nc.scalar.activation(mu4, s4, AF.Copy, scale=1.0 / C)
            msq4 = headp.tile([4, 1], F32, tag="msq4")
            nc.scalar.activation(msq4, sq4, AF.Copy, scale=1.0 / C)
            mu24 = headp.tile([4, 1], F32, tag="mu24")
            nc.vector.tensor_tensor(mu24, mu4, mu4, ALU.mult)
            var4 = headp.tile([4, 1], F32, tag="var4")
            nc.vector.tensor_tensor(var4, msq4, mu24, ALU.subtract)
            sd4 = headp.tile([4, 1], F32, tag="sd4")
            nc.scalar.activation(sd4, var4, AF.Sqrt, bias=eps_f[:])
            rstd4 = headp.tile([4, 1], F32, tag="rstd4")
            nc.vector.reciprocal(rstd4, sd4)
            nc.vector.tensor_scalar(x4, x4, mu4, rstd4, ALU.subtract, ALU.mult)
            lnf = consts.tile([128, 6, 2], F32)
            nc.sync.dma_start(out=lnf, in_=di["lnfp"].ap())
            xhT = headp.tile([128, 6, 4], B16, tag="xhT")
            for ct in range(6):
                tp = psum.tile([128, 4], F32, tag="apsum")
                nc.tensor.transpose(tp, x4[:, ct * 128:(ct + 1) * 128], ident_f4[:])
                nc.scalar.activation(xhT[:, ct, :], tp, AF.Identity,
                                     scale=lnf[:, ct, 0:1], bias=lnf[:, ct, 1:2])
            n0 = 0
            while n0 < VS:
                nn = min(512, VS - n0)
                hp = psum.tile([4, 512], F32, tag="proj")
                for kt in range(6):
                    wt = stream.tile([128, 512], B16, tag="wtile")
                    nc.sync.dma_start(out=wt[:, :nn], in_=di["wteT"].ap()[:, kt, n0:n0 + nn])
                    nc.tensor.matmul(hp[:, :nn], xhT[:, kt, :], wt[:, :nn],
                                     start=(kt == 0), stop=(kt == 5))
                out_t = headp.tile([4, 512], F32, tag="out_t")
                nc.scalar.activation(out_t[:, :nn], hp[:, :nn], AF.Copy)
                nc.sync.dma_start(out=logits.ap()[:, n0:n0 + nn], in_=out_t[:, :nn])
                n0 += nn

    nc.compile()
    return nc


last_exec_time_ns = None


def kernel(**inputs):
    global last_exec_time_ns
    from concourse.bass_utils import run_bass_kernel_spmd

    in_maps = _host_pack(inputs)
    if "nc" not in _g_cache:
        _g_cache["nc"] = _build()
    nc = _g_cache["nc"]

    trace = bool(os.environ.get("BASS_KERNEL_TRACE"))
    res = run_bass_kernel_spmd(nc, in_maps, list(range(NCORES)), trace=trace)
    last_exec_time_ns = res.exec_time_ns
    logits = np.concatenate([res.results[i]["logits"] for i in range(NCORES)], axis=1)
    return logits.reshape(B, 1, V).astype(np.float32)


# revision 19
# speedup vs baseline: 1.0035x; 1.0035x over previous
"""nn_GPT_64347200029289 — RWKV6-style transformer on 8 trn2 cores.

Sharding: DP=4 over batch x TP=2 over heads / FFN. core = 2*batch + rank.
Activations feature-major [C, T] on-chip; per-pair AllReduce after Wo and
after cmix; lm_head sharded over V across all 8 cores via AllGather of the
final token.
"""
import os
import numpy as np
import ml_dtypes

C, H, L, V = 768, 12, 12, 50304
N = C // H          # 64
B, T, Q = 4, 1024, 256
NCH = T // Q        # 4 chunks
GN_EPS = 1e-5 * 64
LN_EPS = 1e-5
NCORES = 8
HPC = H // 2        # heads per core (6)
FS = HPC * N        # 384 tmix shard
F3 = 3 * C // 2     # 1152 cmix shard
VS = V // NCORES    # 6288
PAIRS = [[0, 1], [2, 3], [4, 5], [6, 7]]
GATHER_GROUPS = [[0, 2, 4, 6], [1, 3, 5, 7]]
BF16 = np.float16
AB16 = ml_dtypes.bfloat16  # attention-block dtype

# ---- params column layout (per-partition f32 [128, PCOLS]) ----
_c = 0
def _cols(n):
    global _c
    s = _c
    _c += n
    return s
IMAA = _cols(24)    # (mix 4)x(ct 6): 1-maa
AMAA = _cols(24)    # maa
ICMA = _cols(12)    # (mix 2)x(ct 6): 1-cmaa
ACMA = _cols(12)
BR = _cols(3)
BK = _cols(3)
BCK = _cols(9)
BCR = _cols(6)
BO = _cols(6)
BCV = _cols(6)
GNW = _cols(3)
WB = _cols(12)      # (ih 2)x(h 6)
WK = _cols(12)      # (jh 2)x(h 6)
WS = _cols(6)       # rows 0..63
PCOLS = _c

_g_cache = {}


def _host_pack(inputs):
    f = lambda a: np.ascontiguousarray(np.asarray(a, np.float32))
    idx = np.asarray(inputs["idx"]).astype(np.int64)
    wte, wpe = f(inputs["wte"]), f(inputs["wpe"])
    for k in ("bv", "bg", "gn_b", "bcv", "bck", "ln1_b", "ln2_b"):
        assert np.allclose(np.asarray(inputs[k]), 0.0), f"{k} must be 0"

    Wr, Wk_, Wv, Wg = f(inputs["Wr"]), f(inputs["Wk"]), f(inputs["Wv"]), f(inputs["Wg"])
    Wo, Wck, Wcv, Wcr = f(inputs["Wo"]), f(inputs["Wck"]), f(inputs["Wcv"]), f(inputs["Wcr"])

    td, tf = f(inputs["tdecay"]), f(inputs["tfaaaa"])  # [L,H]
    w = np.exp(-np.exp(td.astype(np.float64)))
    ii = np.arange(Q)
    diff = ii[:, None] - ii[None, :]
    e = np.where(diff > 0, diff - 1, 0).astype(np.float64)
    Wmat = np.where(diff[None, None] > 0, w[:, :, None, None] ** e[None, None],
                    np.where(diff[None, None] == 0, tf[:, :, None, None].astype(np.float64), 0.0))
    WmatT = Wmat.transpose(0, 1, 3, 2)                  # [L,H,j,i]
    wk_d = w[:, :, None] ** (Q - 1 - ii)[None, None, :]  # [L,H,Q]
    wb_d = w[:, :, None] ** ii[None, None, :]            # [L,H,Q]
    ws_d = (w ** Q).astype(np.float32)                   # [L,H]

    per_rank = {}
    for rank in range(2):
        hs = slice(rank * FS, (rank + 1) * FS)
        fs = slice(rank * F3, (rank + 1) * F3)
        heads = list(range(rank * HPC, (rank + 1) * HPC))

        wrk = np.empty((L, 128, 6, 2 * FS), BF16)
        wvg = np.empty((L, 128, 6, 2 * FS), BF16)
        wo = np.empty((L, 128, 3, C), AB16)
        wck = np.empty((L, 128, 6, F3), BF16)
        wcv = np.empty((L, 128, 9, C), BF16)
        wcr = np.empty((L, 128, 6, C), BF16)
        wmt = np.empty((L, HPC, 128, 2, Q), AB16)
        params = np.zeros((L, 128, PCOLS), np.float32)
        prow = np.zeros((L, 1, 4 * C), np.float32)
        for l in range(L):
            for kt in range(6):
                rows = slice(kt * 128, (kt + 1) * 128)
                wrk[l, :, kt, :FS] = Wr[l][rows, hs]
                wrk[l, :, kt, FS:] = Wk_[l][rows, hs]
                wvg[l, :, kt, :FS] = Wv[l][rows, hs]
                wvg[l, :, kt, FS:] = Wg[l][rows, hs]
                wck[l, :, kt, :] = Wck[l][rows, fs]
                wcr[l, :, kt, :] = Wcr[l][rows, :]
            for kt in range(3):
                wo[l, :, kt, :] = Wo[l][rank * FS + kt * 128:rank * FS + (kt + 1) * 128, :]
            for kt in range(9):
                wcv[l, :, kt, :] = Wcv[l][rank * F3 + kt * 128:rank * F3 + (kt + 1) * 128, :]
            for hi, h in enumerate(heads):
                for jh in range(2):
                    wmt[l, hi, :, jh, :] = WmatT[l, h, jh * 128:(jh + 1) * 128, :]
            maas = [f(inputs["maa_tk"])[l], f(inputs["maa_tv"])[l],
                    f(inputs["maa_tr"])[l], f(inputs["maa_tg"])[l]]
            cmaas = [f(inputs["cmaa_k"])[l], f(inputs["cmaa_r"])[l]]
            for mi in range(4):
                for ct in range(6):
                    col = maas[mi][ct * 128:(ct + 1) * 128]
                    params[l, :, IMAA + mi * 6 + ct] = 1.0 - col
                    params[l, :, AMAA + mi * 6 + ct] = col
            for mi in range(2):
                for ct in range(6):
                    col = cmaas[mi][ct * 128:(ct + 1) * 128]
                    params[l, :, ICMA + mi * 6 + ct] = 1.0 - col
                    params[l, :, ACMA + mi * 6 + ct] = col
            br, bk = f(inputs["br"])[l][hs], f(inputs["bk"])[l][hs]
            for ft in range(3):
                params[l, :, BR + ft] = br[ft * 128:(ft + 1) * 128]
                params[l, :, BK + ft] = bk[ft * 128:(ft + 1) * 128]
                params[l, :, GNW + ft] = f(inputs["gn_w"])[l][hs][ft * 128:(ft + 1) * 128]
            bck = f(inputs["bck"])[l][fs]
            for ft in range(9):
                params[l, :, BCK + ft] = bck[ft * 128:(ft + 1) * 128]
            bcr = f(inputs["bcr"])[l]
            bo = f(inputs["bo"])[l] if rank == 0 else np.zeros(C, np.float32)
            bcv = f(inputs["bcv"])[l] if rank == 0 else np.zeros(C, np.float32)
            for ct in range(6):
                params[l, :, BCR + ct] = bcr[ct * 128:(ct + 1) * 128]
                params[l, :, BO + ct] = bo[ct * 128:(ct + 1) * 128]
                params[l, :, BCV + ct] = bcv[ct * 128:(ct + 1) * 128]
            for hi, h in enumerate(heads):
                for ih in range(2):
                    params[l, :, WB + ih * 6 + hi] = wb_d[l, h, ih * 128:(ih + 1) * 128]
                    params[l, :, WK + ih * 6 + hi] = wk_d[l, h, ih * 128:(ih + 1) * 128]
                params[l, :, WS + hi] = ws_d[l, h]
            prow[l, 0, 0 * C:1 * C] = f(inputs["ln1_w"])[l]
            prow[l, 0, 1 * C:2 * C] = f(inputs["ln1_b"])[l]
            prow[l, 0, 2 * C:3 * C] = f(inputs["ln2_w"])[l]
            prow[l, 0, 3 * C:4 * C] = f(inputs["ln2_b"])[l]
        per_rank[rank] = dict(wrk=wrk, wvg=wvg, wo=wo, wck=wck, wcv=wcv,
                              wcr=wcr, wmt=wmt, params=params, prow=prow)

    lnfp = np.zeros((128, 6, 2), np.float32)
    for ct in range(6):
        lnfp[:, ct, 0] = f(inputs["lnf_w"])[ct * 128:(ct + 1) * 128]
        lnfp[:, ct, 1] = f(inputs["lnf_b"])[ct * 128:(ct + 1) * 128]

    in_maps = []
    for core in range(NCORES):
        b, rank = core // 2, core % 2
        x0 = (wte[idx[b]] + wpe[:T]).T                  # [C, T] f32
        x0p = np.zeros((128, 6, T + 1), np.float32)
        for ct in range(6):
            x0p[:, ct, 1:] = x0[ct * 128:(ct + 1) * 128]
        wteT = np.empty((128, 6, VS), BF16)
        vsl = slice(core * VS, (core + 1) * VS)
        for kt in range(6):
            wteT[:, kt, :] = wte[vsl, kt * 128:(kt + 1) * 128].T
        m = dict(per_rank[rank])
        m["x0"] = x0p
        m["wteT"] = wteT
        m["lnfp"] = lnfp
        in_maps.append(m)
    return in_maps


def _build():
    import concourse.bass as bass
    import concourse.mybir as mybir
    import concourse.tile as tile
    from concourse import bacc
    from concourse.masks import make_identity
    from contextlib import ExitStack

    dt = mybir.dt
    AF = mybir.ActivationFunctionType
    ALU = mybir.AluOpType
    F32, B16 = dt.float32, dt.float16
    A16 = dt.bfloat16

    nc = bacc.Bacc("TRN2", target_bir_lowering=False, debug=False,
                   num_devices=NCORES)
    di = {}
    def dram_in(name, shape, dtype):
        di[name] = nc.dram_tensor(name, list(shape), dtype, kind="ExternalInput")
        return di[name]

    dram_in("x0", [128, 6, T + 1], F32)
    dram_in("wrk", [L, 128, 6, 2 * FS], B16)
    dram_in("wvg", [L, 128, 6, 2 * FS], B16)
    dram_in("wo", [L, 128, 3, C], A16)
    dram_in("wck", [L, 128, 6, F3], B16)
    dram_in("wcv", [L, 128, 9, C], B16)
    dram_in("wcr", [L, 128, 6, C], B16)
    dram_in("wmt", [L, HPC, 128, 2, Q], A16)
    dram_in("params", [L, 128, PCOLS], F32)
    dram_in("prow", [L, 1, 4 * C], F32)
    dram_in("wteT", [128, 6, VS], B16)
    dram_in("lnfp", [128, 6, 2], F32)
    logits = nc.dram_tensor("logits", [4, VS], F32, kind="ExternalOutput")

    with tile.TileContext(nc) as tc:
        with ExitStack() as ctx:
            consts = ctx.enter_context(tc.tile_pool(name="consts", bufs=1))
            resid = ctx.enter_context(tc.tile_pool(name="resid", bufs=1))
            wbig = ctx.enter_context(tc.tile_pool(name="wbig", bufs=1))
            wpc = ctx.enter_context(tc.tile_pool(name="wpc", bufs=2))
            lnp = ctx.enter_context(tc.tile_pool(name="lnp", bufs=1))
            stream = ctx.enter_context(tc.tile_pool(name="stream", bufs=2))
            mixp = ctx.enter_context(tc.tile_pool(name="mixp", bufs=2))
            projp = ctx.enter_context(tc.tile_pool(name="projp", bufs=5))
            att = ctx.enter_context(tc.tile_pool(name="att", bufs=2))
            attk = ctx.enter_context(tc.tile_pool(name="attk", bufs=2))
            statep = ctx.enter_context(tc.tile_pool(name="statep", bufs=1))
            smalls = ctx.enter_context(tc.tile_pool(name="smalls", bufs=2))
            lnst = ctx.enter_context(tc.tile_pool(name="lnst", bufs=1))
            xxp = ctx.enter_context(tc.tile_pool(name="xxp", bufs=12))
            h2p = ctx.enter_context(tc.tile_pool(name="h2p", bufs=1))
            headp = ctx.enter_context(tc.tile_pool(name="headp", bufs=1))
            psum = ctx.enter_context(tc.tile_pool(name="psum", bufs=4, space="PSUM"))
            psum1 = ctx.enter_context(tc.tile_pool(name="psum1", bufs=1, space="PSUM"))
            dram = ctx.enter_context(tc.tile_pool(name="dram", bufs=2, space="DRAM"))

            ones_bf = consts.tile([128, 1], B16)
            nc.vector.memset(ones_bf, 1.0)
            ones_row = consts.tile([1, 512], F32)
            nc.vector.memset(ones_row, 1.0)
            eps_ln = consts.tile([1, 1], F32)
            nc.vector.memset(eps_ln, LN_EPS)
            eps_gn = consts.tile([128, 1], F32)
            nc.vector.memset(eps_gn, GN_EPS)
            eps_f = consts.tile([4, 1], F32)
            nc.vector.memset(eps_f, LN_EPS)
            ident_bf = consts.tile([128, 128], A16)
            make_identity(nc, ident_bf[:])
            ident_f4 = consts.tile([4, 4], F32)
            make_identity(nc, ident_f4[:])

            x = resid.tile([128, 6, T + 1], F32)
            nc.sync.dma_start(out=x, in_=di["x0"].ap())

            def layer_norm(pm, l, seg, nmix, imaa0, amaa0):
                """LN of x -> xln [128,6,T+1] bf16 (col0=0) + nmix mix tiles."""
                pr = wpc.tile([1, 2 * C], F32, tag="prow")
                nc.sync.dma_start(out=pr, in_=di["prow"].ap()[l, :, seg * 2 * C:(seg + 1) * 2 * C])
                goff, boff = 0, C
                xln = lnp.tile([128, 6, T + 1], B16, tag="xln")
                nc.vector.memset(xln[:, :, 0:1], 0.0)
                stats = psum1.tile([65, 2, 512], F32, tag="stats")  # sum@p0, sumsq@p64
                for kt in range(6):
                    for tt in range(2):
                        ts1 = slice(1 + tt * 512, 1 + (tt + 1) * 512)
                        xbf = stream.tile([128, 512], B16, tag="xbf")
                        xsq = stream.tile([128, 512], B16, tag="xsq")
                        nc.scalar.activation(xbf, x[:, kt, ts1], AF.Copy)
                        nc.scalar.activation(xsq, x[:, kt, ts1], AF.Square)
                        nc.tensor.matmul(stats[0:1, tt, :], ones_bf, xbf,
                                         start=(kt == 0), stop=(kt == 5))
                        nc.tensor.matmul(stats[64:65, tt, :], ones_bf, xsq,
                                         start=(kt == 0), stop=(kt == 5))
                rstd = lnst.tile([1, T], F32, tag="rstd")
                nmurs = lnst.tile([1, T], F32, tag="nmurs")
                for tt in range(2):
                    ts = slice(tt * 512, (tt + 1) * 512)
                    mu_r = smalls.tile([1, 512], F32, tag="st_a")
                    nc.scalar.activation(mu_r, stats[0:1, tt, :], AF.Copy, scale=1.0 / C)
                    msq_r = smalls.tile([1, 512], F32, tag="st_b")
                    nc.scalar.activation(msq_r, stats[64:65, tt, :], AF.Copy, scale=1.0 / C)
                    mu2 = smalls.tile([1, 512], F32, tag="st_a")
                    nc.vector.tensor_tensor(mu2, mu_r, mu_r, ALU.mult)
                    var = smalls.tile([1, 512], F32, tag="st_b")
                    nc.vector.tensor_tensor(var, msq_r, mu2, ALU.subtract)
                    sd = smalls.tile([1, 512], F32, tag="st_b")
                    nc.scalar.activation(sd, var, AF.Sqrt, bias=eps_ln[:])
                    nc.vector.reciprocal(rstd[:, ts], sd)
                    mrs = smalls.tile([1, 512], F32, tag="st_b")
                    nc.vector.tensor_tensor(mrs, mu_r, rstd[:, ts], ALU.mult)
                    nc.vector.tensor_scalar(nmurs[:, ts], mrs, -1.0, None, ALU.mult)
                for ct in range(6):
                    for tt in range(2):
                        ts = slice(tt * 512, (tt + 1) * 512)
                        ts1 = slice(1 + tt * 512, 1 + (tt + 1) * 512)
                        bca = psum1.tile([128, 512], F32, tag="bca")
                        nc.tensor.matmul(bca, pr[0:1, goff + ct * 128:goff + (ct + 1) * 128],
                                         rstd[:, ts], start=True, stop=True)
                        bcb = psum1.tile([128, 512], F32, tag="bcb")
                        nc.tensor.matmul(bcb, pr[0:1, goff + ct * 128:goff + (ct + 1) * 128],
                                         nmurs[:, ts], start=True, stop=True)
                        nc.vector.tensor_tensor(xln[:, ct, ts1], x[:, ct, ts1], bca, ALU.mult)
                        nc.vector.tensor_tensor(xln[:, ct, ts1], xln[:, ct, ts1], bcb, ALU.add)
                xxs = {}
                for ct in range(6):
                    for tt in range(2):
                        xx = xxp.tile([128, 512], B16, tag="xx")
                        nc.vector.tensor_tensor(
                            xx, xln[:, ct, tt * 512:(tt + 1) * 512],
                            xln[:, ct, 1 + tt * 512:1 + (tt + 1) * 512], ALU.subtract)
                        xxs[(ct, tt)] = xx
                mixes = []
                for mi in range(nmix):
                    mx = mixp.tile([128, 6, T], B16, tag="mix")
                    for ct in range(6):
                        for tt in range(2):
                            nc.vector.scalar_tensor_tensor(
                                mx[:, ct, tt * 512:(tt + 1) * 512], xxs[(ct, tt)],
                                pm[:, amaa0 + mi * 6 + ct:amaa0 + mi * 6 + ct + 1],
                                xln[:, ct, 1 + tt * 512:1 + (tt + 1) * 512], ALU.mult, ALU.add)
                    mixes.append(mx)
                return mixes

            for l in range(L):
                pm = wpc.tile([128, PCOLS], F32, tag="params")
                nc.sync.dma_start(out=pm, in_=di["params"].ap()[l])

                # ---------------- tmix ----------------
                xk, xv, xr, xg = layer_norm(pm, l, 0, 4, IMAA, AMAA)

                rT = projp.tile([128, 3, T], A16, tag="proj")
                kT = projp.tile([128, 3, T], A16, tag="proj")
                for dst, mx, woff, boff in ((kT, xk, FS, BK), (rT, xr, 0, BR)):
                    for ft in range(3):
                        wp = wpc.tile([128, 6, 128], B16, tag="wtm")
                        nc.sync.dma_start(
                            out=wp, in_=di["wrk"].ap()[l, :, :, woff + ft * 128:woff + (ft + 1) * 128])
                        for tt in range(2):
                            ts = slice(tt * 512, (tt + 1) * 512)
                            ps = psum.tile([128, 512], F32, tag="proj")
                            for kt in range(6):
                                nc.tensor.matmul(ps, wp[:, kt, :], mx[:, kt, ts],
                                                 start=(kt == 0), stop=(kt == 5))
                            nc.scalar.activation(dst[:, ft, ts], ps, AF.Identity,
                                                 bias=pm[:, boff + ft:boff + ft + 1])
                v = projp.tile([128, 8, FS], A16, tag="proj")
                g = projp.tile([128, 8, FS], A16, tag="proj")
                w_vg = wbig.tile([128, 6, 2 * FS], B16, tag="wvg")
                nc.sync.dma_start(out=w_vg, in_=di["wvg"].ap()[l])
                for dst, mx, woff, fn in ((v, xv, 0, AF.Copy), (g, xg, FS, AF.Silu)):
                    for pt in range(8):
                        ps = psum.tile([128, FS], F32, tag="proj")
                        for kt in range(6):
                            nc.tensor.matmul(ps, mx[:, kt, pt * 128:(pt + 1) * 128],
                                             w_vg[:, kt, woff:woff + FS],
                                             start=(kt == 0), stop=(kt == 5))
                        nc.scalar.activation(dst[:, pt, :], ps, fn)

                # attention
                state_f = statep.tile([128, HPC // 2, N], F32, tag="state_f")
                state_b = statep.tile([128, HPC // 2, N], A16, tag="state_b")
                y = projp.tile([128, 8, FS], A16, tag="proj")
                for hi in range(HPC):
                    wm = att.tile([128, 2, Q], A16, tag="wmt")
                    nc.sync.dma_start(out=wm, in_=di["wmt"].ap()[l, hi])
                    po = (hi % 2) * 64
                    fq = hi // 2
                    for c in range(NCH):
                        cs = slice(c * Q, (c + 1) * Q)
                        att_ps = psum.tile([128, 2, Q], F32, tag="proj")
                        for jh in range(2):
                            nc.tensor.matmul(
                                att_ps[:, jh],
                                kT[po:po + 64, fq, c * Q + jh * 128:c * Q + (jh + 1) * 128],
                                rT[po:po + 64, fq, cs], start=True, stop=True)
                        aw = attk.tile([128, 2, Q], A16, tag="aw")
                        nc.vector.tensor_tensor(aw, att_ps, wm, ALU.mult)
                        for ih in range(2):
                            pt = c * 2 + ih
                            y1 = psum.tile([128, N], F32, tag="proj")
                            for jh in range(2):
                                nc.tensor.matmul(y1, aw[:, jh, ih * 128:(ih + 1) * 128],
                                                 v[:, c * 2 + jh, hi * N:(hi + 1) * N],
                                                 start=(jh == 0), stop=(jh == 1))
                            if c == 0:
                                nc.scalar.activation(y[:, pt, hi * N:(hi + 1) * N], y1, AF.Copy)
                            else:
                                y2 = psum.tile([128, N], F32, tag="proj")
                                nc.tensor.matmul(
                                    y2, rT[po:po + 64, fq, c * Q + ih * 128:c * Q + (ih + 1) * 128],
                                    state_b[po:po + 64, fq, :], start=True, stop=True)
                                y2s = attk.tile([128, N], F32, tag="y2s")
                                nc.vector.tensor_scalar(y2s, y2, pm[:, WB + ih * 6 + hi:WB + ih * 6 + hi + 1],
                                                        None, ALU.mult)
                                nc.vector.tensor_tensor(y[:, pt, hi * N:(hi + 1) * N],
                                                        y1, y2s, ALU.add)
                        if c < NCH - 1:
                            kwks = []
                            for jh in range(2):
                                tr = psum.tile([128, N], A16, tag="proj")
                                nc.tensor.transpose(
                                    tr, kT[po:po + 64, fq, c * Q + jh * 128:c * Q + (jh + 1) * 128],
                                    ident_bf[po:po + 64, po:po + 64])
                                kwk = attk.tile([128, N], A16, tag="kwk")
                                nc.vector.tensor_scalar(kwk, tr, pm[:, WK + jh * 6 + hi:WK + jh * 6 + hi + 1],
                                                        None, ALU.mult)
                                kwks.append(kwk)
                            st_ps = psum.tile([64, N], F32, tag="proj")
                            for jh in range(2):
                                nc.tensor.matmul(st_ps, kwks[jh], v[:, c * 2 + jh, hi * N:(hi + 1) * N],
                                                 start=(jh == 0), stop=(jh == 1))
                            if c == 0:
                                nc.scalar.activation(state_f[po:po + 64, fq, :], st_ps, AF.Copy)
                            else:
                                sdec = attk.tile([128, N], F32, tag="sdec")
                                nc.vector.tensor_scalar(sdec[po:po + 64], state_f[po:po + 64, fq, :],
                                                        pm[po:po + 64, WS + hi:WS + hi + 1], None, ALU.mult)
                                nc.vector.tensor_tensor(state_f[po:po + 64, fq, :], sdec[po:po + 64], st_ps, ALU.add)
                            nc.scalar.activation(state_b[po:po + 64, fq, :], state_f[po:po + 64, fq, :], AF.Copy)

                # groupnorm (token-major) + gate
                for pt in range(8):
                    y3 = y[:, pt, :].rearrange("p (h n) -> p h n", h=HPC)
                    ysq = stream.tile([128, HPC, N], A16, tag="ysq")
                    nc.scalar.activation(ysq, y3, AF.Square)
                    ssum = smalls.tile([128, HPC], F32, tag="ssum")
                    nc.vector.tensor_reduce(ssum, y3, mybir.AxisListType.X, ALU.add)
                    ssq = smalls.tile([128, HPC], F32, tag="ssq")
                    nc.vector.tensor_reduce(ssq, ysq, mybir.AxisListType.X, ALU.add)
                    mu = smalls.tile([128, HPC], F32, tag="gmu")
                    nc.scalar.activation(mu, ssum, AF.Copy, scale=1.0 / N)
                    msq = smalls.tile([128, HPC], F32, tag="gmsq")
                    nc.scalar.activation(msq, ssq, AF.Copy, scale=1.0 / N)
                    mu2 = smalls.tile([128, HPC], F32, tag="gmu2")
                    nc.vector.tensor_tensor(mu2, mu, mu, ALU.mult)
                    var = smalls.tile([128, HPC], F32, tag="gvar")
                    nc.vector.tensor_tensor(var, msq, mu2, ALU.subtract)
                    sd = smalls.tile([128, HPC], F32, tag="gsd")
                    nc.scalar.activation(sd, var, AF.Sqrt, bias=eps_gn[:])
                    rstd = smalls.tile([128, HPC], F32, tag="grstd")
                    nc.vector.reciprocal(rstd, sd)
                    for hi in range(HPC):
                        nc.vector.tensor_scalar(y[:, pt, hi * N:(hi + 1) * N],
                                                y[:, pt, hi * N:(hi + 1) * N],
                                                mu[:, hi:hi + 1], rstd[:, hi:hi + 1],
                                                ALU.subtract, ALU.mult)
                    nc.vector.tensor_tensor(y[:, pt, :], y[:, pt, :], g[:, pt, :], ALU.mult)

                # transpose yg -> ygT with gn_w fold
                ygT = mixp.tile([128, 3, T], A16, tag="mix")
                for ft in range(3):
                    for pt in range(8):
                        tp = psum.tile([128, 128], A16, tag="proj")
                        nc.tensor.transpose(tp, y[:, pt, ft * 128:(ft + 1) * 128], ident_bf[:])
                        nc.scalar.activation(ygT[:, ft, pt * 128:(pt + 1) * 128], tp,
                                             AF.Copy, scale=pm[:, GNW + ft:GNW + ft + 1])

                # Wo partial + AllReduce
                ar_ins = [dram.tile([128, 6, 512], B16, tag="ar_in", name=f"ari{_t}") for _t in range(2)]
                ar_outs = [dram.tile([128, 6, 512], B16, tag="ar_out", name=f"aro{_t}") for _t in range(2)]
                for tt in range(2):
                    ts = slice(tt * 512, (tt + 1) * 512)
                    for ct in range(6):
                        wp = wpc.tile([128, 3, 128], A16, tag="wo")
                        nc.sync.dma_start(out=wp, in_=di["wo"].ap()[l, :, :, ct * 128:(ct + 1) * 128])
                        ps = psum.tile([128, 512], F32, tag="proj")
                        for kt in range(3):
                            nc.tensor.matmul(ps, wp[:, kt, :], ygT[:, kt, ts],
                                             start=(kt == 0), stop=(kt == 2))
                        ot = stream.tile([128, 512], B16, tag="arin")
                        nc.scalar.activation(ot, ps, AF.Identity, bias=pm[:, BO + ct:BO + ct + 1])
                        nc.sync.dma_start(out=ar_ins[tt][:, ct, :], in_=ot)
                    nc.gpsimd.collective_compute(
                        "AllReduce", ALU.add, replica_groups=PAIRS,
                        ins=[ar_ins[tt].opt()], outs=[ar_outs[tt].opt()])
                for tt in range(2):
                    ts1 = slice(1 + tt * 512, 1 + (tt + 1) * 512)
                    for ct in range(6):
                        art = stream.tile([128, 512], B16, tag="arb")
                        nc.sync.dma_start(out=art, in_=ar_outs[tt][:, ct, :])
                        nc.vector.tensor_tensor(x[:, ct, ts1], x[:, ct, ts1], art, ALU.add)

                # ---------------- cmix ----------------
                xk2, xr2 = layer_norm(pm, l, 1, 2, ICMA, ACMA)
                h2 = h2p.tile([128, 9, T], B16, tag="h2")
                for ft in range(9):
                    wp = wpc.tile([128, 6, 128], B16, tag="wck")
                    nc.sync.dma_start(out=wp, in_=di["wck"].ap()[l, :, :, ft * 128:(ft + 1) * 128])
                    for tt in range(2):
                        ts = slice(tt * 512, (tt + 1) * 512)
                        ps = psum.tile([128, 512], F32, tag="proj")
                        for kt in range(6):
                            nc.tensor.matmul(ps, wp[:, kt, :], xk2[:, kt, ts],
                                             start=(kt == 0), stop=(kt == 5))
                        hr = stream.tile([128, 512], B16, tag="hr")
                        nc.scalar.activation(hr, ps, AF.Relu)
                        nc.vector.tensor_tensor(h2[:, ft, ts], hr, hr, ALU.mult)
                ar_ins2 = [dram.tile([128, 6, 512], B16, tag="ar_in", name=f"ari2{_t}") for _t in range(2)]
                ar_outs2 = [dram.tile([128, 6, 512], B16, tag="ar_out", name=f"aro2{_t}") for _t in range(2)]
                for tt in range(2):
                    ts = slice(tt * 512, (tt + 1) * 512)
                    for ct in range(6):
                        wpr = wpc.tile([128, 6, 128], B16, tag="wcr")
                        nc.sync.dma_start(out=wpr, in_=di["wcr"].ap()[l, :, :, ct * 128:(ct + 1) * 128])
                        wpv = wpc.tile([128, 9, 128], B16, tag="wcv")
                        nc.sync.dma_start(out=wpv, in_=di["wcv"].ap()[l, :, :, ct * 128:(ct + 1) * 128])
                        gp = psum.tile([128, 512], F32, tag="proj")
                        for kt in range(6):
                            nc.tensor.matmul(gp, wpr[:, kt, :], xr2[:, kt, ts],
                                             start=(kt == 0), stop=(kt == 5))
                        gt = stream.tile([128, 512], B16, tag="gate")
                        nc.scalar.activation(gt, gp, AF.Sigmoid, bias=pm[:, BCR + ct:BCR + ct + 1])
                        cp = psum.tile([128, 512], F32, tag="proj")
                        for kt in range(9):
                            nc.tensor.matmul(cp, wpv[:, kt, :], h2[:, kt, ts],
                                             start=(kt == 0), stop=(kt == 8))
                        ot = stream.tile([128, 512], B16, tag="arin")
                        nc.vector.tensor_tensor(ot, cp, gt, ALU.mult)
                        nc.sync.dma_start(out=ar_ins2[tt][:, ct, :], in_=ot)
                    nc.gpsimd.collective_compute(
                        "AllReduce", ALU.add, replica_groups=PAIRS,
                        ins=[ar_ins2[tt].opt()], outs=[ar_outs2[tt].opt()])
                for tt in range(2):
                    ts1 = slice(1 + tt * 512, 1 + (tt + 1) * 512)
                    for ct in range(6):
                        art = stream.tile([128, 512], B16, tag="arb")
                        nc.sync.dma_start(out=art, in_=ar_outs2[tt][:, ct, :])
                        nc.vector.tensor_tensor(x[:, ct, ts1], x[:, ct, ts1], art, ALU.add)

            # ---------------- head ----------------
            xl_d = dram.tile([6, 128], F32, tag="xl")
            for ct in range(6):
                nc.sync.dma_start(out=xl_d[ct], in_=x[:, ct, T])
            xg_d = dram.tile([4, C], F32, tag="xg")
            nc.gpsimd.collective_compute(
                "AllGather", mybir.AluOpType.bypass, replica_groups=GATHER_GROUPS,
                ins=[xl_d.opt()], outs=[xg_d.opt()])
            x4 = headp.tile([4, C], F32, tag="x4")
            nc.sync.dma_start(out=x4, in_=xg_d[:])
            s4 = headp.tile([4, 1], F32, tag="s4")
            nc.vector.tensor_reduce(s4, x4, mybir.AxisListType.X, ALU.add)
            trash = headp.tile([4, C], B16, tag="out_t")
            sq4 = headp.tile([4, 1], F32, tag="sq4")
            nc.scalar.activation(trash, x4, AF.Square, accum_out=sq4)
            mu4 = headp.tile([4, 1], F32, tag="mu4")
            nc.scalar.activation(mu4, s4, AF.Copy, scale=1.0 / C)
            msq4 = headp.tile([4, 1], F32, tag="msq4")
            nc.scalar.activation(msq4, sq4, AF.Copy, scale=1.0 / C)
            mu24 = headp.tile([4, 1], F32, tag="mu24")
            nc.vector.tensor_tensor(mu24, mu4, mu4, ALU.mult)
            var4 = headp.tile([4, 1], F32, tag="var4")
            nc.vector.tensor_tensor(var4, msq4, mu24, ALU.subtract)
            sd4 = headp.tile([4, 1], F32, tag="sd4")
            nc.scalar.activation(sd4, var4, AF.Sqrt, bias=eps_f[:])
            rstd4 = headp.tile([4, 1], F32, tag="rstd4")
            nc.vector.reciprocal(rstd4, sd4)
            nc.vector.tensor_scalar(x4, x4, mu4, rstd4, ALU.subtract, ALU.mult)
            lnf = consts.tile([128, 6, 2], F32)
            nc.sync.dma_start(out=lnf, in_=di["lnfp"].ap())
            xhT = headp.tile([128, 6, 4], B16, tag="xhT")
            for ct in range(6):
                tp = psum.tile([128, 4], F32, tag="proj")
                nc.tensor.transpose(tp, x4[:, ct * 128:(ct + 1) * 128], ident_f4[:])
                nc.scalar.activation(xhT[:, ct, :], tp, AF.Identity,
                                     scale=lnf[:, ct, 0:1], bias=lnf[:, ct, 1:2])
            n0 = 0
            while n0 < VS:
                nn = min(512, VS - n0)
                hp = psum.tile([4, 512], F32, tag="proj")
                for kt in range(6):
                    wt = stream.tile([128, 512], B16, tag="wtile")
                    nc.sync.dma_start(out=wt[:, :nn], in_=di["wteT"].ap()[:, kt, n0:n0 + nn])
                    nc.tensor.matmul(hp[:, :nn], xhT[:, kt, :], wt[:, :nn],
                                     start=(kt == 0), stop=(kt == 5))
                out_t = headp.tile([4, 512], F32, tag="out_t")
                nc.scalar.activation(out_t[:, :nn], hp[:, :nn], AF.Copy)
                nc.sync.dma_start(out=logits.ap()[:, n0:n0 + nn], in_=out_t[:, :nn])
                n0 += nn

    nc.compile()
    return nc


last_exec_time_ns = None


def kernel(**inputs):
    global last_exec_time_ns
    from concourse.bass_utils import run_bass_kernel_spmd

    in_maps = _host_pack(inputs)
    if "nc" not in _g_cache:
        _g_cache["nc"] = _build()
    nc = _g_cache["nc"]

    trace = bool(os.environ.get("BASS_KERNEL_TRACE"))
    res = run_bass_kernel_spmd(nc, in_maps, list(range(NCORES)), trace=trace)
    last_exec_time_ns = res.exec_time_ns
    logits = np.concatenate([res.results[i]["logits"] for i in range(NCORES)], axis=1)
    return logits.reshape(B, 1, V).astype(np.float32)
